# revision 42
# baseline (speedup 1.0000x reference)
"""Trainium2 Bass kernel for nn_Attention_Block (B=4, S=1024, D=1024, H=16).

Self-contained: `kernel(**inputs)` takes FULL inputs, shards across 8
NeuronCores internally (batch x query-half), returns FULL outputs
(y, att_scores).

Per-core plan (core ci -> batch b=ci//2, query rows half=ci%2):
  - S^T = (k/8) @ q^T per head via f32r matmuls (kv on partitions) -> exp on
    ScalarE (bf16) -> AV matmul with a per-head ones-column fused into v to
    produce softmax denominators for free.
  - S (normal orientation, q on partitions) recomputed via f32r matmuls and
    streamed out as the att_scores output (PSUM -> SBUF copy -> HBM DMA).
  - ctx normalized by reciprocal denominators broadcast across partitions
    (2KB partition-shift DMA + GpSimd partition_broadcast), proj/minus as
    f32r matmuls, LayerNorm fused on the minus PSUM output.
"""

import numpy as np

B, S, D, H, DH = 4, 1024, 1024, 16, 64
SQ = S // 2
NCORES = 8
EPS = 1e-5

_CACHE = {}


def _get_built():
    if "nc" not in _CACHE:
        _CACHE["nc"] = _build()
    return _CACHE["nc"]


def _emit_sn(nc, p, ncopy, ps_sn, snsb, qT_sb, kT_sb, att_o, F32, ts, ds,
             irange=range(4)):
    for i in irange:
        sos = []
        for e in (0, 1):
            sos.append(snsb.tile([128, 1024], F32, tag="sn_sb", name="so"))
        for j2 in range(2):
            sns = []
            for e in (0, 1):
                off = e * 64
                sn = ps_sn.tile([128, 512], F32, tag="sn", name="sn")
                nc.tensor.matmul(
                    sn[:],
                    qT_sb[off:off + 64, p, ts(i, 128)],
                    kT_sb[off:off + 64, p, ds(j2 * 512, 512)],
                    start=True, stop=True,
                )
                sns.append(sn)
            for e in (0, 1):
                so = sos[e]
                if ncopy % 3 == 2:
                    nc.scalar.copy(so[:, ds(j2 * 512, 512)], sns[e][:])
                else:
                    nc.vector.tensor_copy(so[:, ds(j2 * 512, 512)], sns[e][:])
                ncopy += 1
        for e in (0, 1):
            h = 2 * p + e
            nc.sync.dma_start(att_o[h, ts(i, 128), :], sos[e][:])
    return ncopy


def _build():
    from contextlib import ExitStack

    import concourse.tile as tile
    from concourse import bacc, mybir
    from concourse.bass import ds, ts

    F32 = mybir.dt.float32
    F32R = mybir.dt.float32r
    BF16 = mybir.dt.bfloat16
    AF = mybir.ActivationFunctionType
    AX = mybir.AxisListType
    OP = mybir.AluOpType

    nc = bacc.Bacc("TRN2", target_bir_lowering=False, debug=False,
                   num_devices=NCORES)

    qT = nc.dram_tensor("qT", (D, SQ), F32R, kind="ExternalInput").ap()
    kT = nc.dram_tensor("kT", (D, S), F32R, kind="ExternalInput").ap()
    vaug = nc.dram_tensor("vaug", (S, H * 72), BF16, kind="ExternalInput").ap()
    wpT = nc.dram_tensor("wpT", (D, D), F32R, kind="ExternalInput").ap()
    wmT = nc.dram_tensor("wmT", (2 * D, D), F32R, kind="ExternalInput").ap()
    lnw = nc.dram_tensor("lnw", (1, D), F32, kind="ExternalInput").ap()
    lnb = nc.dram_tensor("lnb", (1, D), F32, kind="ExternalInput").ap()
    att_o = nc.dram_tensor("att_o", (H, SQ, S), F32, kind="ExternalOutput").ap()
    y_o = nc.dram_tensor("y_o", (SQ, D), F32, kind="ExternalOutput").ap()

    with tile.TileContext(nc) as tc, ExitStack() as ctx:
        const = ctx.enter_context(tc.tile_pool(name="const", bufs=1))

        qT_sb = const.tile([128, 8, SQ], F32R, tag="qT_sb")
        kT_sb = const.tile([128, 8, S], F32R, tag="kT_sb")
        v_sb = const.tile([128, 8, H * 72], BF16, tag="v_sb")
        qT_r = qT.rearrange("(c p) q -> p c q", p=128)
        kT_r = kT.rearrange("(c p) s -> p c s", p=128)
        v_r = vaug.rearrange("(c p) w -> p c w", p=128)
        for cch in range(8):
            nc.sync.dma_start(qT_sb[:, cch, :], qT_r[:, cch, :])
            nc.sync.dma_start(kT_sb[:, cch, :], kT_r[:, cch, :])
            nc.sync.dma_start(v_sb[:, cch, :], v_r[:, cch, :])
        lnw_row = const.tile([1, D], F32, tag="lnw_row")
        nc.sync.dma_start(lnw_row[:], lnw)
        lnb_row = const.tile([1, D], F32, tag="lnb_row")
        nc.sync.dma_start(lnb_row[:], lnb)
        lnw_sb = const.tile([128, D], F32, tag="lnw_sb")
        nc.gpsimd.partition_broadcast(lnw_sb[:], lnw_row[0:1, :])
        lnb_sb = const.tile([128, D], F32, tag="lnb_sb")
        nc.gpsimd.partition_broadcast(lnb_sb[:], lnb_row[0:1, :])

        ctxu = const.tile([128, 8, SQ], F32R, tag="ctxu")  # ctx^T, head pairs
        ctxo = const.tile([64, 8, SQ], F32R, tag="ctxo")   # odd-head staging
        xT_sb = const.tile([128, 8, SQ], F32R, tag="xT_sb")

        ncopy = 0  # round-robin S-norm copies across DVE/ACT

        with (
            tc.tile_pool(name="denp", bufs=3) as denp,
            tc.tile_pool(name="tinyp", bufs=3) as tinyp,
            tc.tile_pool(name="bcp", bufs=3) as bcp,
            tc.tile_pool(name="ps_st", bufs=2, space="PSUM") as ps_st,
            tc.tile_pool(name="ps_av", bufs=2, space="PSUM") as ps_av,
            tc.tile_pool(name="ps_sn", bufs=2, space="PSUM") as ps_sn,  # last => top banks
            tc.tile_pool(name="expp", bufs=3) as expp,
            tc.tile_pool(name="snsb", bufs=8) as snsb,
        ):
            for p in range(8):
                # transposed scores + exp; heads 2p (partitions 0:64) and
                # 2p+1 (64:128) interleaved so K=64 matmuls pack into
                # distinct PE row-groups. st tiles are 2-bank [128,1024];
                # one exp covers two kv-chunks.
                ets = [
                    expp.tile([128, 8, 512], BF16, tag="expS",
                              name=f"expS_{2 * p}"),
                    expp.tile([128, 8, 512], BF16, tag="expS",
                              name=f"expS_{2 * p + 1}"),
                ]
                for e in (0, 1):
                    off = e * 64
                    for t in range(4):
                        st = ps_st.tile([128, 1024], F32, tag="st")
                        for jj in (0, 1):
                            j = 2 * t + jj
                            nc.tensor.matmul(
                                st[:, ds(jj * 512, 512)],
                                kT_sb[off:off + 64, p, ts(j, 128)],
                                qT_sb[off:off + 64, p, :],
                                start=True, stop=True,
                            )
                        nc.scalar.activation(
                            ets[e][:, ds(2 * t, 2), :].rearrange(
                                "p a b -> p (a b)"),
                            st[:], AF.Exp)
                    if p > 0:
                        ncopy = _emit_sn(nc, p - 1, ncopy, ps_sn, snsb,
                                         qT_sb, kT_sb, att_o, F32, ts, ds,
                                         irange=range(2 * e, 2 * e + 2))
                # AV chains (K=128, bf16) with fused ones-column (col 64)
                avs = []
                for e in (0, 1):
                    h = 2 * p + e
                    av = ps_av.tile([65, 512], F32, tag="av")
                    for j in range(8):
                        nc.tensor.matmul(
                            av[0:65, :],
                            v_sb[:, j, h * 72:h * 72 + 65],
                            ets[e][:, j, :],
                            start=(j == 0), stop=(j == 7),
                        )
                    avs.append(av)
                for e in (0, 1):
                    h = 2 * p + e
                    av = avs[e]
                    # denominator: reciprocal straight off PSUM row 64, shift
                    # to partition 0 via 2KB DMA, broadcast on GpSimd, then
                    # normalize directly out of PSUM into ctxu (bf16).
                    dh = denp.tile([65, SQ], F32, tag="den", name=f"den_{h}")
                    nc.vector.reciprocal(dh[64:65, :], av[64:65, :])
                    # drain ctx out of PSUM immediately so the av slot frees;
                    # normalization happens later, SBUF-only.
                    if e == 0:
                        nc.vector.tensor_copy(ctxu[0:64, p, :], av[0:64, :])
                    else:
                        nc.vector.tensor_copy(ctxo[:, p, :], av[0:64, :])
                    tn = tinyp.tile([1, SQ], F32, tag="tiny", name=f"tn_{h}")
                    nc.sync.dma_start(tn[:], dh[64:65, :])
                    bch = bcp.tile([64, SQ], F32, tag="bc", name=f"bc_{h}")
                    nc.gpsimd.partition_broadcast(bch[:], tn[0:1, :])
                    if e == 0:
                        nc.vector.tensor_tensor(ctxu[0:64, p, :],
                                                ctxu[0:64, p, :],
                                                bch[:], op=OP.mult)
                    else:
                        nc.vector.tensor_tensor(ctxo[:, p, :], ctxo[:, p, :],
                                                bch[:], op=OP.mult)
                        # shift odd head to partitions 64:128 of the pair
                        nc.sync.dma_start(ctxu[64:128, p, :], ctxo[:, p, :])
            ncopy = _emit_sn(nc, 7, ncopy, ps_sn, snsb, qT_sb, kT_sb, att_o,
                             F32, ts, ds)

        # proj + minus in one PSUM epoch: ps_y (banks 0-6, chunks 0-2)
        # opens first so the minus q-half can start as soon as the attention
        # st/av pools drain, concurrent with proj; ps_x (banks 6-8) is scoped
        # to proj only, then y3 reuses those banks.
        with (
            tc.tile_pool(name="wpp", bufs=3) as wpp,
            tc.tile_pool(name="wmp", bufs=6) as wmp,
            tc.tile_pool(name="ps_y", bufs=1, space="PSUM") as ps_y,
            tc.tile_pool(name="lns", bufs=2) as lns,
            tc.tile_pool(name="yout", bufs=2) as yout,
        ):
            yps = [ps_y.tile([128, S], F32, tag=f"y{i}", name=f"yps_{i}")
                   for i in range(3)]
            with tc.tile_pool(name="ps_x", bufs=2, space="PSUM") as ps_x:
                for do in range(8):
                    wp = wpp.tile([128, 8, 128], F32R, tag="wp")
                    # wp[p, c, n] = W_proj.T[c*128+p, do*128+n]
                    nc.sync.dma_start(
                        wp[:],
                        wpT.rearrange("(c p) n -> p c n", p=128)[:, :, ts(do, 128)],
                    )
                    xp = ps_x.tile([128, 512], F32, tag="xp")
                    for c in range(8):
                        nc.tensor.matmul(
                            xp[:], wp[:, c, :], ctxu[:, c, :],
                            start=(c == 0), stop=(c == 7),
                        )
                    nc.vector.tensor_copy(xT_sb[:, do, :], xp[:])
            ps_y2 = ctx.enter_context(
                tc.tile_pool(name="ps_y2", bufs=1, space="PSUM"))
            yps.append(ps_y2.tile([128, S], F32, tag="y3", name="yps_3"))
            wmT_r = wmT.rearrange("(c p) n -> p c n", p=128)
            for cic in range(16):
                wm = wmp.tile([128, D], F32R, tag="wm")
                nc.sync.dma_start(wm[:], wmT_r[:, cic, :])
                for i in range(4):
                    if cic < 8:
                        lhsT = qT_sb[:, cic, ts(i, 128)]
                    else:
                        lhsT = xT_sb[:, cic - 8, ts(i, 128)]
                    for j2 in range(2):
                        nc.tensor.matmul(
                            yps[i][:, ds(j2 * 512, 512)],
                            lhsT,
                            wm[:, ds(j2 * 512, 512)],
                            start=(cic == 0), stop=(cic == 15),
                        )
            inv = 1.0 / D
            for i in range(4):
                nmu = lns.tile([128, 1], F32, tag="nmu")
                if i >= 2:
                    sscr = lns.tile([128, S], F32, tag="sscr")
                    nc.scalar.activation(sscr[:], yps[i][:], AF.Identity,
                                         accum_out=nmu[:])
                    nc.vector.tensor_scalar_mul(nmu[:], nmu[:], -inv)
                else:
                    nc.vector.reduce_sum(nmu[:], yps[i][:], axis=AX.X,
                                         negate=True)
                    nc.vector.tensor_scalar_mul(nmu[:], nmu[:], inv)
                sq = lns.tile([128, S], F32, tag="sq")
                ssum = lns.tile([128, 1], F32, tag="ssum")
                nc.scalar.activation(sq[:], yps[i][:], AF.Square,
                                     accum_out=ssum[:])
                nc.vector.tensor_scalar_mul(ssum[:], ssum[:], inv)
                musq = lns.tile([128, 1], F32, tag="musq")
                nc.vector.tensor_tensor(musq[:], nmu[:], nmu[:], op=OP.mult)
                nc.vector.tensor_tensor(ssum[:], ssum[:], musq[:],
                                        op=OP.subtract)
                nc.vector.tensor_scalar_add(ssum[:], ssum[:], EPS)  # var+eps
                srt = lns.tile([128, 1], F32, tag="srt")
                nc.scalar.sqrt(srt[:], ssum[:])
                r0 = lns.tile([128, 1], F32, tag="r0")
                nc.vector.reciprocal(r0[:], srt[:])
                # one Newton step: rs = r0 * (1.5 - 0.5 * vpe * r0^2)
                t1 = lns.tile([128, 1], F32, tag="t1")
                nc.vector.tensor_tensor(t1[:], r0[:], r0[:], op=OP.mult)
                nc.vector.tensor_tensor(t1[:], t1[:], ssum[:], op=OP.mult)
                nc.vector.tensor_scalar(t1[:], t1[:], -0.5, 1.5,
                                        op0=OP.mult, op1=OP.add)
                nc.vector.tensor_tensor(t1[:], t1[:], r0[:], op=OP.mult)
                ysb = yout.tile([128, S], F32, tag="ysb")
                if i < 2:
                    nmurs = lns.tile([128, 1], F32, tag="nmurs")
                    nc.vector.tensor_tensor(nmurs[:], nmu[:], t1[:], op=OP.mult)
                    nc.scalar.activation(ysb[:], yps[i][:], AF.Identity,
                                         bias=nmurs[:], scale=t1[:])
                else:
                    nc.vector.tensor_scalar(ysb[:], yps[i][:], nmu[:], t1[:],
                                            op0=OP.add, op1=OP.mult)
                eng = nc.vector if i >= 2 else nc.gpsimd
                eng.tensor_tensor(ysb[:], ysb[:], lnw_sb[:], op=OP.mult)
                eng.tensor_tensor(ysb[:], ysb[:], lnb_sb[:], op=OP.add)
                nc.sync.dma_start(y_o[ts(i, 128), :], ysb[:])

    nc.compile()
    return nc


def _numpy_ref(q, k, v, mask, scores, W_proj, W_minus, ln_w, ln_b, c):
    """Exact numpy fallback for general inputs (c != 0 or mask != 1)."""
    qh = q.reshape(B, S, H, DH)
    kh = k.reshape(B, S, H, DH)
    vh = v.reshape(B, S, H, DH)
    att_scores = (
        np.einsum("bqhd,bkhd->bhqk", qh, kh).astype(np.float32) / np.sqrt(DH)
        + c[0] * scores
    )
    att_scores = att_scores - 1e8 * (1.0 - mask)[:, None, None, :]
    m = att_scores.max(axis=-1, keepdims=True)
    e = np.exp(att_scores - m)
    att = e / e.sum(axis=-1, keepdims=True)
    ctx = np.einsum("bhqk,bkhd->bqhd", att, vh).reshape(B, S, D)
    x = ctx @ W_proj.T
    cat = np.concatenate([q, x], axis=-1)
    y = cat @ W_minus.T
    mu = y.mean(axis=-1, keepdims=True)
    var = ((y - mu) ** 2).mean(axis=-1, keepdims=True)
    y = (y - mu) / np.sqrt(var + EPS) * ln_w + ln_b
    return (y.astype(np.float32), att_scores.astype(np.float32))


def kernel(q, k, v, mask, scores, W_proj, W_minus, ln_w, ln_b, c):
    f32 = np.float32
    q = np.asarray(q, dtype=f32)
    k = np.asarray(k, dtype=f32)
    v = np.asarray(v, dtype=f32)
    mask = np.asarray(mask, dtype=f32)
    W_proj = np.asarray(W_proj, dtype=f32)
    W_minus = np.asarray(W_minus, dtype=f32)
    ln_w = np.asarray(ln_w, dtype=f32)
    ln_b = np.asarray(ln_b, dtype=f32)
    c = np.asarray(c, dtype=f32)

    if not (np.all(c == 0.0) and np.all(mask == 1.0)):
        scores = np.asarray(scores, dtype=f32)
        return _numpy_ref(q, k, v, mask, scores, W_proj, W_minus, ln_w, ln_b, c)

    import ml_dtypes

    try:
        import jax
        jax.config.update("jax_compilation_cache_dir", "/tmp/jax_cc_cache")
        jax.config.update("jax_persistent_cache_min_entry_size_bytes", -1)
        jax.config.update("jax_persistent_cache_min_compile_time_secs", 0.0)
    except Exception:
        pass

    from concourse.bass_utils import run_bass_kernel_spmd

    nc = _get_built()

    wpT = np.ascontiguousarray(W_proj.T)
    wmT = np.ascontiguousarray(W_minus.T)
    lnw = np.ascontiguousarray(ln_w.reshape(1, D))
    lnb = np.ascontiguousarray(ln_b.reshape(1, D))

    in_maps = []
    for b in range(B):
        kTb = np.ascontiguousarray(k[b].T) * f32(1.0 / np.sqrt(DH))
        vb = v[b]
        va = np.zeros((S, H, 72), dtype=ml_dtypes.bfloat16)
        va[:, :, 0:DH] = vb.reshape(S, H, DH).astype(ml_dtypes.bfloat16)
        for h in range(H):
            va[:, h, DH] = 1.0
        va = np.ascontiguousarray(va.reshape(S, H * 72))
        qTb = np.ascontiguousarray(q[b].T)
        for half in range(2):
            in_maps.append(dict(
                qT=np.ascontiguousarray(qTb[:, half * SQ:(half + 1) * SQ]),
                kT=kTb, vaug=va, wpT=wpT, wmT=wmT, lnw=lnw, lnb=lnb,
            ))

    res = run_bass_kernel_spmd(nc, in_maps, core_ids=list(range(NCORES)))
    _CACHE["last_results"] = res

    att = np.empty((B, H, S, S), f32)
    y = np.empty((B, S, D), f32)
    for ci in range(NCORES):
        b, half = divmod(ci, 2)
        r = res.results[ci]
        att[b, :, half * SQ:(half + 1) * SQ, :] = r["att_o"]
        y[b, half * SQ:(half + 1) * SQ, :] = r["y_o"]
    return (y, att)
